# revision 17
# baseline (speedup 1.0000x reference)
"""Trainium2 Bass kernel for nn_CyberMoE: MHA gating + MoE routing.

Strategy: data-parallel over batch across 8 NeuronCores (32 batches/core).
All compute in fp32 (top-2 expert selection margins are ~1e-5 in
gating_probs, so reduced-precision matmuls flip expert selections).

Key algebraic restructurings (exact in real arithmetic):
  - seq_repr = mean_s(ao @ WoT + bo) = (mean_s ao) @ WoT + bo
    -> out-projection runs on 32 mean vectors instead of 4096 tokens.
  - mean-over-queries of attention output per (batch, head):
      ao_mean = v.T @ colsum,  colsum = exp.T @ recip_rowsums
    (softmax normalization folded into the column-sum matmul).
  - v.T @ colsum = Wv @ (x.T @ colsum): compress tokens FIRST (u = x.T@cs,
    one [128,768]x[128,8] matmul per batch), then apply Wv once to the 256
    compressed vectors -> the entire V projection GEMM disappears.
  - sum_j colsum_j == S exactly, so the v-bias contribution is constant and
    b_out' = b_out + b_v @ w_out.T is folded host-side.
  - 1/sqrt(HD) folded into Wq; 1/S folded into Wo (host-side).
  - softmax max-subtraction dropped: scores for this model/input family are
    bounded (measured |s| < 2), exp cannot overflow in fp32.
  - stage-2 GEMMs run activation-stationary (weights are the moving
    operand, N up to 512) so tiny-N matmul overhead disappears, and
    outputs land directly in [batch, feature] layout for LayerNorm.
"""

import os
import numpy as np
from contextlib import ExitStack

import concourse.bass as bass
import concourse.mybir as mybir
import concourse.tile as tile
from concourse import bacc
from concourse.bass import ts
from concourse.bass_utils import run_bass_kernel_spmd

F32 = mybir.dt.float32
AF = mybir.ActivationFunctionType
ALU = mybir.AluOpType
AX = mybir.AxisListType

B, S, H, E, L, K, NH = 256, 128, 768, 5, 2, 2, 8
HD = H // NH  # 96
NCORES = 8
BC = B // NCORES          # 32 batches per core
T = BC * S                # 4096 tokens per core
NBLK = 8                  # token blocks per core
BLK = T // NBLK           # 512 tokens per block
BPB = BLK // S            # 4 batches per block
KH = H // 128             # 6 k-tiles over H
LN_EPS = 1e-5

DBG_NBLK = int(os.environ.get("CYBERMOE_NBLK", NBLK))


# Output columns of W_qk (and W_v) are host-permuted so that tile j (of 6)
# holds head j's 96 dims at rows 0:96 and chunk (j%3) of head 6+(j//3) at
# rows 96:128.  All SBUF partition accesses then satisfy the HW rule
# (start in {0,32,64,96}; 32/96-start spans <= 32).
def _qk_perm():
    perm = []
    for j in range(6):
        perm.extend(range(96 * j, 96 * j + 96))
        h = 6 + j // 3
        c = j % 3
        perm.extend(range(96 * h + 32 * c, 96 * h + 32 * c + 32))
    return np.array(perm, dtype=np.int64)


def _qk_copies(j):
    """Copies for permuted tile j: (src_row0, nrows, head, dst_row0)."""
    h = 6 + j // 3
    c = j % 3
    return [(0, 96, j, 0), (96, 32, h, 32 * c)]


def _ln_gelu(nc, pool, x_sb, D, g_sb, be_sb, eps_sb):
    """LayerNorm over free axis + exact GELU on a [BC, D] sbuf tile."""
    ssum = pool.tile([BC, 1], F32, tag="s2stat", bufs=12, name="ssum")
    negmean = pool.tile([BC, 1], F32, tag="s2stat", bufs=12, name="negmean")
    nc.vector.reduce_sum(ssum[:], x_sb[:], axis=AX.X)
    nc.scalar.mul(negmean[:], ssum[:], -1.0 / D)
    xm = pool.tile([BC, D], F32, tag="s2act", bufs=5, name="xm")
    nc.vector.tensor_scalar_add(xm[:], x_sb[:], negmean[:])
    sq = pool.tile([BC, D], F32, tag="s2act", bufs=5, name="sq")
    vsum = pool.tile([BC, 1], F32, tag="s2stat", bufs=12, name="vsum")
    nc.scalar.activation(sq[:], xm[:], AF.Square, accum_out=vsum[:])
    std = pool.tile([BC, 1], F32, tag="s2stat", bufs=12, name="std")
    nc.scalar.activation(std[:], vsum[:], AF.Sqrt, bias=eps_sb[:], scale=1.0 / D)
    rstd = pool.tile([BC, 1], F32, tag="s2stat", bufs=12, name="rstd")
    nc.vector.reciprocal(rstd[:], std[:])
    xn = pool.tile([BC, D], F32, tag="s2act", bufs=5, name="xn")
    nc.vector.tensor_scalar_mul(xn[:], xm[:], rstd[:])
    y = pool.tile([BC, D], F32, tag="s2act", bufs=5, name="y")
    nc.vector.tensor_mul(y[:], xn[:], g_sb[:])
    y2 = pool.tile([BC, D], F32, tag="s2act", bufs=5, name="y2")
    nc.vector.tensor_add(y2[:], y[:], be_sb[:])
    out = pool.tile([BC, D], F32, tag="s2act", bufs=5, name="lnout")
    nc.scalar.activation(out[:], y2[:], AF.Gelu)
    return out


def build_program():
    nc = bacc.Bacc("TRN2", target_bir_lowering=False, debug=False,
                   enable_asserts=False, num_devices=NCORES)

    def inp(name, shape):
        return nc.declare_dram_parameter(name, list(shape), F32, isOutput=False)

    def outp(name, shape):
        return nc.declare_dram_parameter(name, list(shape), F32, isOutput=True)

    d_xt = inp("xt", (H, T))            # x transposed  [feat, token]
    d_xtok = inp("xtok", (T, H))        # x natural     [token, feat]
    d_clst = inp("clst", (H, BC))
    d_wqkt = inp("wqkt", (H, 2 * H))    # [WqT*scale | WkT], head-permuted
    d_bqk = inp("bqk", (128, 12))
    d_wvt = inp("wvt", (H, H))          # WvT, out-cols head-permuted
    d_wot = inp("wot", (NH, HD, H))     # (Wo/S).T per head
    d_boutb = inp("boutb", (BC, H))     # b_out + b_v @ Wo.T, broadcast
    d_ident = inp("ident", (128, 128))
    d_wf1t = inp("wf1t", (H, 2 * H))
    d_bf1b = inp("bf1b", (BC, 2 * H))
    d_gf1 = inp("gf1", (BC, 2 * H))
    d_bef1 = inp("bef1", (BC, 2 * H))
    d_wf2t = inp("wf2t", (2 * H, 2 * H))
    d_bf2b = inp("bf2b", (BC, 2 * H))
    d_gf2 = inp("gf2", (BC, 2 * H))
    d_bef2 = inp("bef2", (BC, 2 * H))
    d_wct = inp("wct", (2 * H, H))
    d_bcb = inp("bcb", (BC, H))
    d_gc = inp("gc", (BC, H))
    d_bec = inp("bec", (BC, H))
    d_wr1t = inp("wr1t", (H, H // 2))
    d_br1b = inp("br1b", (BC, H // 2))
    d_gr1 = inp("gr1", (BC, H // 2))
    d_ber1 = inp("ber1", (BC, H // 2))
    d_wr2t = inp("wr2t", (H // 2, E))
    d_br2b = inp("br2b", (BC, E))
    d_wd1t = inp("wd1t", (H, H // 2))
    d_bd1b = inp("bd1b", (BC, H // 2))
    d_gd1 = inp("gd1", (BC, H // 2))
    d_bed1 = inp("bed1", (BC, H // 2))
    d_wd2t = inp("wd2t", (H // 2, E))
    d_bd2b = inp("bd2b", (BC, E))
    d_wet = inp("wet", (H, E * L))
    d_beb = inp("beb", (BC, E * L))

    d_final = outp("final", (BC, L))
    d_gating = outp("gating", (BC, E))
    d_expert = outp("expert", (BC, E * L))
    d_domain = outp("domain", (BC, E))

    with tile.TileContext(nc) as tc, ExitStack() as top:
        persist = top.enter_context(tc.tile_pool(name="persist", bufs=1))
        const = top.enter_context(tc.tile_pool(name="const", bufs=1))
        w2a_stack = ExitStack()
        w2a = w2a_stack.enter_context(tc.tile_pool(name="w2a", bufs=1))

        ident = const.tile([128, 128], F32)
        nc.sync.dma_start(ident[:], d_ident[:])
        bqk_sb = const.tile([128, 12], F32)
        nc.sync.dma_start(bqk_sb[:], d_bqk[:])
        eps_sb = const.tile([BC, 1], F32)
        nc.gpsimd.memset(eps_sb[:], LN_EPS)
        clst_sb = [const.tile([128, BC], F32, tag=f"clst{k}", name=f"clst{k}")
                   for k in range(KH)]
        for k in range(KH):
            nc.sync.dma_start(clst_sb[k][:], d_clst[ts(k, 128), :])

        # prefetched stage-2 weights (DMA free to run during stage 1)
        wot_sb = [w2a.tile([128, H], F32, tag=f"wo{h}", name=f"wo{h}")
                  for h in range(NH)]
        for h in range(NH):
            nc.sync.dma_start(wot_sb[h][0:HD, :], d_wot[h])
        wvt_sb = [w2a.tile([128, H], F32, tag=f"wv{k}", name=f"wv{k}")
                  for k in range(KH)]
        for k in range(KH):
            nc.sync.dma_start(wvt_sb[k][:], d_wvt[ts(k, 128), :])
        # attention-mean accumulator [d(96 used), h*32+b] and u accumulator
        seq_ao2 = persist.tile([128, NH * BC], F32)
        seqT_sb = [persist.tile([128, BC], F32, tag=f"seqT{m}", name=f"seqT{m}")
                   for m in range(KH)]
        u_sb = [persist.tile([128, NH * BC], F32, tag=f"u{k}", name=f"u{k}")
                for k in range(KH)]

        # ---------------- Stage 1 ----------------
        with ExitStack() as s1:
            w1 = s1.enter_context(tc.tile_pool(name="w1", bufs=1))
            xpool = s1.enter_context(tc.tile_pool(name="xp", bufs=2))
            strips = s1.enter_context(tc.tile_pool(name="strips", bufs=1))
            epool = s1.enter_context(tc.tile_pool(name="ep", bufs=4))
            stat = s1.enter_context(tc.tile_pool(name="stat", bufs=6))
            gps = s1.enter_context(
                tc.tile_pool(name="gps", bufs=2, space=bass.MemorySpace.PSUM))
            sps = s1.enter_context(
                tc.tile_pool(name="sps", bufs=2, space=bass.MemorySpace.PSUM))
            cps = s1.enter_context(
                tc.tile_pool(name="cps", bufs=2, space=bass.MemorySpace.PSUM))
            ups = s1.enter_context(
                tc.tile_pool(name="ups", bufs=2, space=bass.MemorySpace.PSUM))

            wqkt_sb = [w1.tile([128, 2 * H], F32, tag=f"wqk{k}", name=f"wqk{k}")
                       for k in range(KH)]
            for k in range(KH):
                nc.sync.dma_start(wqkt_sb[k][:], d_wqkt[ts(k, 128), :])

            for blk in range(DBG_NBLK):
                xt_t = [xpool.tile([128, BLK], F32, tag=f"xt{k}", name=f"xt{k}")
                        for k in range(KH)]
                for k in range(KH):
                    nc.sync.dma_start(xt_t[k][:], d_xt[ts(k, 128), ts(blk, BLK)])
                xtok_t = [xpool.tile([128, H], F32, tag=f"xk{bb}", name=f"xk{bb}")
                          for bb in range(BPB)]
                for bb in range(BPB):
                    nc.sync.dma_start(xtok_t[bb][:],
                                      d_xtok[ts(blk * BPB + bb, S), :])

                # q,k projection (head-permuted output rows)
                qh = [strips.tile([128, BLK], F32, tag=f"qh{h}", name=f"qh{h}")
                      for h in range(NH)]
                kh = [strips.tile([128, BLK], F32, tag=f"kh{h}", name=f"kh{h}")
                      for h in range(NH)]
                for m in range(12):
                    ps = gps.tile([128, BLK], F32, tag="gps", name="gps")
                    for k in range(KH):
                        nc.tensor.matmul(
                            ps[:], wqkt_sb[k][:, ts(m, 128)], xt_t[k][:],
                            start=(k == 0), stop=(k == KH - 1))
                    dest = qh if m < 6 else kh
                    for (p0, ln, h, d0) in _qk_copies(m % 6):
                        nc.vector.tensor_scalar_add(
                            dest[h][d0:d0 + ln, :], ps[p0:p0 + ln, :],
                            bqk_sb[p0:p0 + ln, m:m + 1])

                # attention: colsum per (batch, head), then token compression
                for bb in range(BPB):
                    b = blk * BPB + bb
                    cs_b8 = stat.tile([128, NH], F32, tag="cs8", bufs=3,
                                      name="cs8")
                    for h in range(NH):
                        sc = sps.tile([128, S], F32, tag="sc", name="sc")
                        nc.tensor.matmul(sc[:], qh[h][0:HD, ts(bb, S)],
                                         kh[h][0:HD, ts(bb, S)],
                                         start=True, stop=True)
                        ex = epool.tile([128, S], F32, tag="exp", name="ex")
                        rowsum = stat.tile([128, 1], F32, tag="rs", name="rs")
                        nc.scalar.activation(ex[:], sc[:], AF.Exp,
                                             accum_out=rowsum[:])
                        r = stat.tile([128, 1], F32, tag="rcp", name="rcp")
                        nc.vector.reciprocal(r[:], rowsum[:])
                        exn = epool.tile([128, S], F32, tag="exn", name="exn")
                        nc.vector.tensor_scalar_mul(exn[:], ex[:], r[:])
                        exT = cps.tile([128, S], F32, tag="exT", name="exT")
                        nc.tensor.transpose(exT[:], exn[:], ident[:])
                        nc.vector.reduce_sum(cs_b8[:, h:h + 1], exT[:], axis=AX.X)
                    # u_b = x_b.T @ cs_b8 : [feat, 8]
                    for k in range(KH):
                        u_ps = ups.tile([128, NH], F32, tag="ups", name="ups")
                        nc.tensor.matmul(u_ps[:], xtok_t[bb][:, ts(k, 128)],
                                         cs_b8[:], start=True, stop=True)
                        nc.vector.tensor_copy(u_sb[k][:, ts(b, NH)], u_ps[:])

        # ---- vd GEMM + out-projection (within w2a scope) ----
        with ExitStack() as s15:
            e1p = s15.enter_context(tc.tile_pool(name="e1p", bufs=2))
            fps0 = s15.enter_context(
                tc.tile_pool(name="fps0", bufs=2, space=bass.MemorySpace.PSUM))
            tps0 = s15.enter_context(
                tc.tile_pool(name="tps0", bufs=2, space=bass.MemorySpace.PSUM))

            # vd = Wv @ u  -> scatter into seq_ao2 (head-permuted rows)
            for m in range(KH):
                ps = fps0.tile([128, NH * BC], F32, tag="vdps", bufs=2,
                               name="vdps")
                for k in range(KH):
                    nc.tensor.matmul(ps[:], wvt_sb[k][:, ts(m, 128)], u_sb[k][:],
                                     start=(k == 0), stop=(k == KH - 1))
                # columns are ordered b*8+h; head strips want h*32+b
                psv = ps[:].rearrange("p (b h) -> p h b", h=NH)
                sqv = seq_ao2[:].rearrange("p (h b) -> p h b", b=BC)
                for (p0, ln, h, d0) in _qk_copies(m):
                    nc.vector.tensor_copy(sqv[d0:d0 + ln, h, :],
                                          psv[p0:p0 + ln, h, :])

            # out-projection on the 32 mean vectors: seq [32, 768]
            boutb_sb = e1p.tile([BC, H], F32, name="boutb")
            nc.sync.dma_start(boutb_sb[:], d_boutb[:])
            seq_sb = e1p.tile([BC, H], F32, name="seq")
            for c0 in range(0, H, 512):
                cw = min(512, H - c0)
                ps = fps0.tile([BC, 512], F32, tag="fps", name="fps")
                for h in range(NH):
                    nc.tensor.matmul(ps[0:BC, 0:cw],
                                     seq_ao2[0:HD, ts(h, BC)],
                                     wot_sb[h][0:HD, c0:c0 + cw],
                                     start=(h == 0), stop=(h == NH - 1))
                nc.vector.tensor_add(seq_sb[:, c0:c0 + cw], ps[0:BC, 0:cw],
                                     boutb_sb[:, c0:c0 + cw])
            for m in range(KH):
                ps = tps0.tile([128, BC], F32, tag="tps", name="tps")
                nc.tensor.transpose(ps[:], seq_sb[:, ts(m, 128)],
                                    ident[0:BC, 0:BC])
                nc.vector.tensor_copy(seqT_sb[m][:], ps[:])
        w2a_stack.close()

        # ---------------- Stage 2 ----------------
        with ExitStack() as s2:
            s2p = s2.enter_context(tc.tile_pool(name="s2p", bufs=3))
            wpc = s2.enter_context(tc.tile_pool(name="wpc", bufs=1))
            fps = s2.enter_context(
                tc.tile_pool(name="fps", bufs=3, space=bass.MemorySpace.PSUM))
            tps = s2.enter_context(
                tc.tile_pool(name="tps", bufs=3, space=bass.MemorySpace.PSUM))

            def to_cols(x_sb, D, tag):
                """[BC, D] -> list of [128, BC] tiles (transposed)."""
                outs = []
                for m in range(D // 128):
                    ps = tps.tile([128, BC], F32, tag="tps", name="tps")
                    nc.tensor.transpose(ps[:], x_sb[:, ts(m, 128)],
                                        ident[0:BC, 0:BC])
                    o = s2p.tile([128, BC], F32, tag=tag, bufs=12,
                                 name=f"{tag}{m}")
                    nc.vector.tensor_copy(o[:], ps[:])
                    outs.append(o)
                return outs

            def gemm_flip(inT, w_tiles, outfeat, bias_sb, kparts=128):
                """[BC, outfeat] = inT.T @ W  (+ bias), weights moving."""
                out_sb = s2p.tile([BC, outfeat], F32, tag="s2act", bufs=5,
                                  name="gf")
                nk = len(inT)
                for c0 in range(0, outfeat, 512):
                    cw = min(512, outfeat - c0)
                    ps = fps.tile([BC, 512], F32, tag="fps", name="fps")
                    for k in range(nk):
                        nc.tensor.matmul(ps[0:BC, 0:cw],
                                         inT[k][0:kparts, :],
                                         w_tiles[k][0:kparts, c0:c0 + cw],
                                         start=(k == 0), stop=(k == nk - 1))
                    nc.vector.tensor_add(out_sb[:, c0:c0 + cw], ps[0:BC, 0:cw],
                                         bias_sb[:, c0:c0 + cw])
                return out_sb

            def load_w(wp, dram, n_tiles, width, tag):
                tiles = [wp.tile([128, width], F32, tag=f"{tag}{k}",
                                 name=f"{tag}{k}") for k in range(n_tiles)]
                for k in range(n_tiles):
                    nc.sync.dma_start(tiles[k][:], dram[ts(k, 128), :])
                return tiles

            def load_w_chunked(wp, dram, n_tiles, width, tag):
                """Chunk the free dim so each 512-col group DMAs
                independently and pipelines with the chunk GEMMs."""
                out = []
                for k in range(n_tiles):
                    row = []
                    for ci, c0 in enumerate(range(0, width, 512)):
                        cw = min(512, width - c0)
                        t = wp.tile([128, cw], F32, tag=f"{tag}{k}_{ci}",
                                    name=f"{tag}{k}_{ci}")
                        nc.sync.dma_start(t[:], dram[ts(k, 128), c0:c0 + cw])
                        row.append(t)
                    out.append(row)
                return out

            def gemm_flip_c(inT, w_ck, outfeat, bias_sb, kparts=128):
                out_sb = s2p.tile([BC, outfeat], F32, tag="s2act", bufs=5,
                                  name="gfc")
                nk = len(inT)
                for ci, c0 in enumerate(range(0, outfeat, 512)):
                    cw = min(512, outfeat - c0)
                    ps = fps.tile([BC, 512], F32, tag="fps", name="fps")
                    for k in range(nk):
                        nc.tensor.matmul(ps[0:BC, 0:cw],
                                         inT[k][0:kparts, :],
                                         w_ck[k][ci][:],
                                         start=(k == 0), stop=(k == nk - 1))
                    nc.vector.tensor_add(out_sb[:, c0:c0 + cw], ps[0:BC, 0:cw],
                                         bias_sb[:, c0:c0 + cw])
                return out_sb

            def load_b(wp, dram, width, name):
                t = wp.tile([BC, width], F32, name=name)
                nc.sync.dma_start(t[:], dram[:])
                return t

            # domain head
            wd1 = load_w(wpc, d_wd1t, KH, H // 2, "wd1")
            bd1b = load_b(wpc, d_bd1b, H // 2, "bd1b")
            gd1 = load_b(wpc, d_gd1, H // 2, "gd1")
            bed1 = load_b(wpc, d_bed1, H // 2, "bed1")
            d1 = gemm_flip(clst_sb, wd1, H // 2, bd1b)
            d1o = _ln_gelu(nc, s2p, d1, H // 2, gd1, bed1, eps_sb)
            d1oT = to_cols(d1o, H // 2, "d1oT")
            wd2 = load_w(wpc, d_wd2t, 3, E, "wd2")
            bd2b = load_b(wpc, d_bd2b, E, "bd2b")
            domain = gemm_flip(d1oT, wd2, E, bd2b)
            nc.sync.dma_start(d_domain[:], domain[:])

            # experts: all_exp = cls @ w_e.T + b_e  -> [BC, E*L]
            we = load_w(wpc, d_wet, KH, E * L, "we")
            beb = load_b(wpc, d_beb, E * L, "beb")
            all_exp = gemm_flip(clst_sb, we, E * L, beb)

            with ExitStack() as sa:
                wpa = sa.enter_context(tc.tile_pool(name="wpa", bufs=1))
                wf1_sb = load_w_chunked(wpa, d_wf1t, KH, 2 * H, "wf1")
                bf1b = load_b(wpa, d_bf1b, 2 * H, "bf1b")
                gf1 = load_b(wpa, d_gf1, 2 * H, "gf1")
                bef1 = load_b(wpa, d_bef1, 2 * H, "bef1")
                f1 = gemm_flip_c(seqT_sb, wf1_sb, 2 * H, bf1b)
                f1o = _ln_gelu(nc, s2p, f1, 2 * H, gf1, bef1, eps_sb)
                f1oT = to_cols(f1o, 2 * H, "f1oT")

            with ExitStack() as sb:
                wpb = sb.enter_context(tc.tile_pool(name="wpb", bufs=1))
                wf2 = load_w_chunked(wpb, d_wf2t, 12, 2 * H, "wf2")
                bf2b = load_b(wpb, d_bf2b, 2 * H, "bf2b")
                gf2 = load_b(wpb, d_gf2, 2 * H, "gf2")
                bef2 = load_b(wpb, d_bef2, 2 * H, "bef2")
                f2 = gemm_flip_c(f1oT, wf2, 2 * H, bf2b)
                f2o = _ln_gelu(nc, s2p, f2, 2 * H, gf2, bef2, eps_sb)
                f2oT = to_cols(f2o, 2 * H, "f2oTt")

            wpcc = s2.enter_context(tc.tile_pool(name="wpcc", bufs=1))
            wc = load_w_chunked(wpcc, d_wct, 12, H, "wc")
            bcb = load_b(wpcc, d_bcb, H, "bcb")
            gc = load_b(wpcc, d_gc, H, "gc")
            bec = load_b(wpcc, d_bec, H, "bec")
            cx = gemm_flip_c(f2oT, wc, H, bcb)
            co = _ln_gelu(nc, s2p, cx, H, gc, bec, eps_sb)
            coT = to_cols(co, H, "coT")

            wr1 = load_w(wpc, d_wr1t, KH, H // 2, "wr1")
            br1b = load_b(wpc, d_br1b, H // 2, "br1b")
            gr1 = load_b(wpc, d_gr1, H // 2, "gr1")
            ber1 = load_b(wpc, d_ber1, H // 2, "ber1")
            r1 = gemm_flip(coT, wr1, H // 2, br1b)
            r1o = _ln_gelu(nc, s2p, r1, H // 2, gr1, ber1, eps_sb)
            r1oT = to_cols(r1o, H // 2, "r1oT")

            wr2 = load_w(wpc, d_wr2t, 3, E, "wr2")
            br2b = load_b(wpc, d_br2b, E, "br2b")
            rlog = gemm_flip(r1oT, wr2, E, br2b)

            # gating softmax over E=5
            ngmax = s2p.tile([BC, 1], F32, tag="s2stat", bufs=12, name="ngm")
            nc.vector.reduce_max(ngmax[:], rlog[:], axis=AX.X, negate=True)
            gexp = s2p.tile([BC, E], F32, tag="s2small", bufs=20, name="gexp")
            gsum = s2p.tile([BC, 1], F32, tag="s2stat", bufs=12, name="gsum")
            nc.scalar.activation(gexp[:], rlog[:], AF.Exp, bias=ngmax[:],
                                 accum_out=gsum[:])
            ginv = s2p.tile([BC, 1], F32, tag="s2stat", bufs=12, name="ginv")
            nc.vector.reciprocal(ginv[:], gsum[:])
            gating = s2p.tile([BC, E], F32, tag="s2small", bufs=20, name="gat")
            nc.vector.tensor_scalar_mul(gating[:], gexp[:], ginv[:])
            nc.sync.dma_start(d_gating[:], gating[:])

            # top-2 selection (branch-free)
            m1 = s2p.tile([BC, 1], F32, tag="s2stat", bufs=12, name="m1")
            nc.vector.reduce_max(m1[:], gating[:], axis=AX.X)
            ismax = s2p.tile([BC, E], F32, tag="s2small", bufs=20, name="ism")
            nc.vector.tensor_scalar(ismax[:], gating[:], m1[:], None,
                                    op0=ALU.is_ge)
            g2 = s2p.tile([BC, E], F32, tag="s2small", bufs=20, name="g2")
            nc.vector.scalar_tensor_tensor(g2[:], ismax[:], -1e9, gating[:],
                                           op0=ALU.mult, op1=ALU.add)
            m2 = s2p.tile([BC, 1], F32, tag="s2stat", bufs=12, name="m2")
            nc.vector.reduce_max(m2[:], g2[:], axis=AX.X)
            sel = s2p.tile([BC, E], F32, tag="s2small", bufs=20, name="sel")
            nc.vector.tensor_scalar(sel[:], gating[:], m2[:], None,
                                    op0=ALU.is_ge)
            wsum = s2p.tile([BC, 1], F32, tag="s2stat", bufs=12, name="ws")
            nc.vector.tensor_add(wsum[:], m1[:], m2[:])
            winv = s2p.tile([BC, 1], F32, tag="s2stat", bufs=12, name="wi")
            nc.vector.reciprocal(winv[:], wsum[:])
            wsel = s2p.tile([BC, E], F32, tag="s2small", bufs=20, name="wsel")
            nc.vector.tensor_mul(wsel[:], sel[:], gating[:])
            wn = s2p.tile([BC, E], F32, tag="s2small", bufs=20, name="wn")
            nc.vector.tensor_scalar_mul(wn[:], wsel[:], winv[:])

            sel2 = s2p.tile([BC, E * L], F32, tag="s2small", bufs=20,
                            name="sel2")
            wn2 = s2p.tile([BC, E * L], F32, tag="s2small", bufs=20, name="wn2")
            for ll in range(L):
                sel2v = sel2[:].rearrange("p (e l) -> p e l", l=L)
                wn2v = wn2[:].rearrange("p (e l) -> p e l", l=L)
                nc.vector.tensor_copy(sel2v[:, :, ll], sel[:])
                nc.vector.tensor_copy(wn2v[:, :, ll], wn[:])

            expert = s2p.tile([BC, E * L], F32, tag="s2small", bufs=20,
                              name="exprt")
            nc.vector.tensor_mul(expert[:], all_exp[:], sel2[:])
            nc.sync.dma_start(d_expert[:], expert[:])

            wl = s2p.tile([BC, E * L], F32, tag="s2small", bufs=20, name="wl")
            nc.vector.tensor_mul(wl[:], all_exp[:], wn2[:])
            final = s2p.tile([BC, L], F32, tag="s2small", bufs=20, name="fin")
            wlv = wl[:].rearrange("p (e l) -> p l e", l=L)
            nc.vector.reduce_sum(final[:], wlv, axis=AX.X)
            nc.sync.dma_start(d_final[:], final[:])

    nc.compile()
    return nc


_NC_CACHE = None


def _get_nc():
    global _NC_CACHE
    if _NC_CACHE is None:
        _NC_CACHE = build_program()
    return _NC_CACHE


def _prep_inputs(inputs):
    f = lambda a: np.ascontiguousarray(np.asarray(a, dtype=np.float32))

    x = f(inputs["hidden_state"])                       # (B, S, H)
    w_qkv = f(inputs["w_qkv"])
    b_qkv = f(inputs["b_qkv"])
    wq, wk, wv = w_qkv[0:H], w_qkv[H:2 * H], w_qkv[2 * H:3 * H]
    bq, bk, bv = b_qkv[0:H], b_qkv[H:2 * H], b_qkv[2 * H:3 * H]
    scale = 1.0 / np.sqrt(HD)

    perm = _qk_perm()
    wqkt = np.concatenate([(wq.T * scale)[:, perm], wk.T[:, perm]], axis=1)
    bqk_cols = np.concatenate([(bq * scale)[perm], bk[perm]])
    bqk = np.ascontiguousarray(bqk_cols.reshape(12, 128).T)
    wvt = np.ascontiguousarray(wv.T[:, perm])           # (H, H) permuted out

    w_out = f(inputs["w_out"])
    wot = np.ascontiguousarray((w_out / S).T.reshape(NH, HD, H))
    bout_eff = f(inputs["b_out"]) + bv @ w_out.T        # fold v-bias

    def tcol(name):
        return np.ascontiguousarray(f(inputs[name]).T)

    def bcast(arr, D):
        return np.ascontiguousarray(np.broadcast_to(
            np.asarray(arr, np.float32), (BC, D)))

    shared = {
        "wqkt": wqkt, "bqk": bqk, "wvt": wvt,
        "wot": wot, "boutb": bcast(bout_eff, H),
        "ident": np.eye(128, dtype=np.float32),
        "wf1t": tcol("w_f1"), "bf1b": bcast(f(inputs["b_f1"]), 2 * H),
        "gf1": bcast(f(inputs["g_f1"]), 2 * H),
        "bef1": bcast(f(inputs["be_f1"]), 2 * H),
        "wf2t": tcol("w_f2"), "bf2b": bcast(f(inputs["b_f2"]), 2 * H),
        "gf2": bcast(f(inputs["g_f2"]), 2 * H),
        "bef2": bcast(f(inputs["be_f2"]), 2 * H),
        "wct": tcol("w_c"), "bcb": bcast(f(inputs["b_c"]), H),
        "gc": bcast(f(inputs["g_c"]), H),
        "bec": bcast(f(inputs["be_c"]), H),
        "wr1t": tcol("w_r1"), "br1b": bcast(f(inputs["b_r1"]), H // 2),
        "gr1": bcast(f(inputs["g_r1"]), H // 2),
        "ber1": bcast(f(inputs["be_r1"]), H // 2),
        "wr2t": tcol("w_r2"), "br2b": bcast(f(inputs["b_r2"]), E),
        "wd1t": tcol("w_d1"), "bd1b": bcast(f(inputs["b_d1"]), H // 2),
        "gd1": bcast(f(inputs["g_d1"]), H // 2),
        "bed1": bcast(f(inputs["be_d1"]), H // 2),
        "wd2t": tcol("w_d2"), "bd2b": bcast(f(inputs["b_d2"]), E),
        "wet": np.ascontiguousarray(f(inputs["w_e"]).reshape(E * L, H).T),
        "beb": bcast(f(inputs["b_e"]).reshape(E * L), E * L),
    }

    xflat = x.reshape(B * S, H)
    xt_full = np.ascontiguousarray(xflat.T)
    cls_full = np.ascontiguousarray(x[:, 0, :].T)

    in_maps = []
    for c in range(NCORES):
        m = dict(shared)
        m["xt"] = np.ascontiguousarray(xt_full[:, c * T:(c + 1) * T])
        m["xtok"] = np.ascontiguousarray(xflat[c * T:(c + 1) * T, :])
        m["clst"] = np.ascontiguousarray(cls_full[:, c * BC:(c + 1) * BC])
        in_maps.append(m)
    return in_maps


def kernel(**inputs):
    nc = _get_nc()
    in_maps = _prep_inputs(inputs)
    res = run_bass_kernel_spmd(nc, in_maps, list(range(NCORES)))
    results = res.results
    final = np.concatenate([results[c]["final"] for c in range(NCORES)], axis=0)
    gating = np.concatenate([results[c]["gating"] for c in range(NCORES)], axis=0)
    expert = np.concatenate([results[c]["expert"] for c in range(NCORES)], axis=0)
    domain = np.concatenate([results[c]["domain"] for c in range(NCORES)], axis=0)
    return (final, gating, expert.reshape(B, E, L), domain)
